# revision 4
# baseline (speedup 1.0000x reference)
"""Trainium2 Bass kernel for nn_MultiHeadAttention_39135742001649.

Reference computation (B=2, S=2048, D=1024, H=16, WIN=512):
    q/k/v = x @ W.T + b (per-head dk=64)
    scores = q k^T / 8                               [B,H,S,S]
    probs1 = blockwise softmax: causal mask, softmax within each 512-wide
             column block (masked entries -> 0)
    probs2 = full-row softmax(probs1)  (no masking; exp(0)=1 entries!)
    out    = (probs2 @ v) @ Wo.T + bo

Decomposition used here (validated to 8e-7 rel err vs reference in fp32):
    e1   = exp(scores) * tril_mask        (only 10 of 16 causal blocks)
    d1   = colsum of e1 within block      -> probs1 = e1 / d1
    e2   = exp(probs1)                    (masked/uncomputed entries -> 1)
    out_row = (sum_causal e2 @ v + suffix_colsum_v) / (sum_causal e2 + 512*(3-bi))

Sharding: 8 cores = 2 batches x 4 head-groups (4 heads each). Each core
computes q^T/k^T/v for its heads, the attention, and a partial output
projection over its 256 d-rows; the host sums the 4 partials per batch.

All on-chip layouts are transposed ([c, q] / [d, s]) so matmul contraction
is on partitions. Matmuls run as float32r (full PE rate at free dim >= 256,
fp32 numerics).
"""

import numpy as np
from contextlib import ExitStack

import concourse.bass as bass
import concourse.mybir as mybir
import concourse.tile as tile
from concourse import bacc
from concourse.bass_utils import run_bass_kernel_spmd

F32 = mybir.dt.float32
F32R = mybir.dt.float32r
EXP = mybir.ActivationFunctionType.Exp
ADD = mybir.AluOpType.add
MULT = mybir.AluOpType.mult

B, S, D, H, WIN = 2, 2048, 1024, 16, 512
DK = D // H          # 64
NB = S // WIN        # 4
NCORES = 8
HPC = 4              # heads per core
DCORE = HPC * DK     # 256
P = 128

TRACE = False        # set True from test.py to capture HW profile
TRACE_CORES = None

_CACHE = {}


def _mm(nc, out, lhsT, rhs, start, stop):
    nc.tensor.matmul(out, lhsT, rhs, start=start, stop=stop)


def build_nc():
    nc = bacc.Bacc("TRN2", target_bir_lowering=False, debug=False)

    xT = nc.dram_tensor("xT", [D, S], F32R, kind="ExternalInput")        # x[b].T
    wqT = nc.dram_tensor("wqT", [D, DCORE], F32R, kind="ExternalInput")  # (Wq/8).T slice
    wkT = nc.dram_tensor("wkT", [D, DCORE], F32R, kind="ExternalInput")
    wvT = nc.dram_tensor("wvT", [D, DCORE], F32R, kind="ExternalInput")
    woT = nc.dram_tensor("woT", [DCORE, D], F32R, kind="ExternalInput")  # Wo.T row slice
    bq = nc.dram_tensor("bq", [DCORE], F32, kind="ExternalInput")       # /8
    bk = nc.dram_tensor("bk", [DCORE], F32, kind="ExternalInput")
    bvr = nc.dram_tensor("bvr", [P, DCORE], F32, kind="ExternalInput")  # bv replicated
    maskd = nc.dram_tensor("maskd", [NB, P, WIN], F32, kind="ExternalInput")
    onesd = nc.dram_tensor("onesd", [P, P], F32R, kind="ExternalInput")
    sfxd = nc.dram_tensor("sfxd", [P, 2, NB], F32, kind="ExternalInput")
    outT = nc.dram_tensor("outT", [D, S], F32, kind="ExternalOutput")   # partial out^T

    with tile.TileContext(nc) as tc, ExitStack() as ctx:
        const = ctx.enter_context(tc.tile_pool(name="const", bufs=1))
        wpool = ctx.enter_context(tc.tile_pool(name="wpool", bufs=1))
        persist = ctx.enter_context(tc.tile_pool(name="persist", bufs=1))

        ones128 = const.tile([P, P], F32R, name="ones128")
        nc.sync.dma_start(ones128[:], onesd[:])
        mask_sb = const.tile([P, NB, WIN], F32, name="mask_sb")
        nc.sync.dma_start(mask_sb[:], maskd[:].rearrange("m p q -> p m q"))
        bq_sb = const.tile([P, 2], F32, name="bq_sb")
        nc.sync.dma_start(bq_sb[:], bq[:].rearrange("(c p) -> p c", p=P))
        bk_sb = const.tile([P, 2], F32, name="bk_sb")
        nc.sync.dma_start(bk_sb[:], bk[:].rearrange("(c p) -> p c", p=P))
        bvr_sb = const.tile([P, DCORE], F32, name="bvr_sb")
        nc.sync.dma_start(bvr_sb[:], bvr[:])

        wq_sb = wpool.tile([P, 8, DCORE], F32R, name="wq_sb")
        nc.sync.dma_start(wq_sb[:], wqT[:].rearrange("(o p) d -> p o d", p=P))
        wk_sb = wpool.tile([P, 8, DCORE], F32R, name="wk_sb")
        nc.sync.dma_start(wk_sb[:], wkT[:].rearrange("(o p) d -> p o d", p=P))
        wv_sb = wpool.tile([P, 8, DCORE], F32R, name="wv_sb")
        nc.sync.dma_start(wv_sb[:], wvT[:].rearrange("(o p) d -> p o d", p=P))
        wo_sb = wpool.tile([P, 2, D], F32R, name="wo_sb")
        nc.sync.dma_start(wo_sb[:], woT[:].rearrange("(o p) e -> p o e", p=P))

        qT_sb = persist.tile([P, 2, S], F32R, name="qT_sb")    # [d%128, d//128, s]
        kT_sb = persist.tile([P, 2, S], F32R, name="kT_sb")
        v_sb = persist.tile([P, 16, DCORE], F32R, name="v_sb")  # [s%128, s//128, d]
        attnT_sb = persist.tile([P, 2, S], F32R, name="attnT_sb")
        sfx_sb = const.tile([P, 2, NB], F32, name="sfx_sb")    # suffix sums (host)
        nc.sync.dma_start(sfx_sb[:], sfxd[:])

        # ---------------- Phase A: projections ----------------
        with (
            tc.tile_pool(name="xp", bufs=1) as xp,
            tc.tile_pool(name="psQK", bufs=3, space="PSUM") as psQK,
            tc.tile_pool(name="psV", bufs=3, space="PSUM") as psV,
        ):
            x_sb = xp.tile([P, 8, S], F32R, name="x_sb")
            xTr = xT[:].rearrange("(o p) s -> p o s", p=P)
            for o in range(8):
                nc.sync.dma_start(x_sb[:, o, :], xTr[:, o, :])

            # q^T and k^T: [DCORE, S] as [128, 2, S]
            for w_sb, b_sb, dst in ((wq_sb, bq_sb, qT_sb), (wk_sb, bk_sb, kT_sb)):
                for dc in range(2):
                    for st in range(NB):
                        ps = psQK.tile([P, WIN], F32, name="qk_ps")
                        for o in range(8):
                            _mm(nc, ps[:], w_sb[:, o, dc * P:(dc + 1) * P],
                                x_sb[:, o, st * WIN:(st + 1) * WIN],
                                start=(o == 0), stop=(o == 7))
                        nc.vector.tensor_scalar_add(
                            dst[:, dc, st * WIN:(st + 1) * WIN], ps[:],
                            b_sb[:, dc:dc + 1])

            # v: [S, DCORE] as [128, 16, DCORE]
            for sc in range(16):
                ps = psV.tile([P, DCORE], F32, name="v_ps")
                for o in range(8):
                    _mm(nc, ps[:], x_sb[:, o, sc * P:(sc + 1) * P], wv_sb[:, o, :],
                        start=(o == 0), stop=(o == 7))
                nc.vector.tensor_tensor(v_sb[:, sc, :], ps[:], bvr_sb[:], ADD)

        # ---------------- Phase B: attention ----------------
        with (
            tc.tile_pool(name="e1p", bufs=2) as e1p,
            tc.tile_pool(name="s2p", bufs=2) as s2p,
            tc.tile_pool(name="e2p", bufs=2) as e2p,
            tc.tile_pool(name="drp", bufs=2) as drp,
            tc.tile_pool(name="d2sp", bufs=2) as d2sp,
            tc.tile_pool(name="psSC", bufs=1, space="PSUM") as psSC,
            tc.tile_pool(name="psD1", bufs=2, space="PSUM") as psD1,
            tc.tile_pool(name="psPV", bufs=1, space="PSUM") as psPV,
            tc.tile_pool(name="psD2", bufs=1, space="PSUM") as psD2,
        ):
            for h in range(HPC):
                hc, hb = h // 2, (h % 2) * DK
                for bi in range(NB):
                    pv_ps = psPV.tile([P, WIN], F32, name="pv_ps")
                    d2_ps = psD2.tile([P, WIN], F32, name="d2_ps")
                    for j in range(bi + 1):
                        sc_ps = psSC.tile([P, NB, WIN], F32, name="sc_ps")
                        for m in range(NB):
                            lhsT = kT_sb[hb:hb + DK, hc,
                                         j * WIN + m * P: j * WIN + (m + 1) * P]
                            rhs = qT_sb[hb:hb + DK, hc, bi * WIN:(bi + 1) * WIN]
                            _mm(nc, sc_ps[:, m, :], lhsT, rhs, start=True, stop=True)
                        e1 = e1p.tile([P, NB, WIN], F32R, name="e1")
                        nc.scalar.activation(e1[:], sc_ps[:], EXP)
                        if j == bi:
                            nc.vector.tensor_tensor(e1[:], e1[:], mask_sb[:], MULT)
                        d1_ps = psD1.tile([P, WIN], F32, name="d1_ps")
                        for m in range(NB):
                            _mm(nc, d1_ps[:], ones128[:], e1[:, m, :],
                                start=(m == 0), stop=(m == 3))
                        d1r = drp.tile([P, WIN], F32, name="d1r")
                        nc.vector.reciprocal(d1r[:], d1_ps[:])
                        s2 = s2p.tile([P, NB, WIN], F32, name="s2")
                        nc.vector.tensor_tensor(
                            s2[:], e1[:],
                            d1r[:, None, :].to_broadcast([P, NB, WIN]), MULT)
                        e2 = e2p.tile([P, NB, WIN], F32R, name="e2")
                        nc.scalar.activation(e2[:], s2[:], EXP)
                        first = (j == 0)
                        last = (j == bi)
                        for m in range(NB):
                            # fp32r needs a 128-col stationary operand: use the
                            # 2-head slice; the other head's rows are junk.
                            _mm(nc, pv_ps[:, :],
                                v_sb[:, j * 4 + m, hc * P:(hc + 1) * P], e2[:, m, :],
                                start=(first and m == 0), stop=(last and m == 3))
                            _mm(nc, d2_ps[:], ones128[:], e2[:, m, :],
                                start=(first and m == 0), stop=(last and m == 3))
                    # fixup: attnT = (pv + sfx) / (d2 + 512*(3-bi))
                    d2s = d2sp.tile([P, WIN], F32, name="d2s")
                    nc.vector.tensor_scalar_add(d2s[hb:hb + DK, :],
                                                d2_ps[hb:hb + DK, :],
                                                float(WIN * (NB - 1 - bi)))
                    nc.vector.reciprocal(d2s[hb:hb + DK, :], d2s[hb:hb + DK, :])
                    nc.vector.scalar_tensor_tensor(
                        attnT_sb[hb:hb + DK, hc, bi * WIN:(bi + 1) * WIN],
                        pv_ps[hb:hb + DK, :],
                        sfx_sb[hb:hb + DK, hc, bi:bi + 1],
                        d2s[hb:hb + DK, :],
                        ADD, MULT)

        # ---------------- Phase C: output projection ----------------
        with (
            tc.tile_pool(name="otp", bufs=3) as otp,
            tc.tile_pool(name="psO", bufs=4, space="PSUM") as psO,
        ):
            for ec in range(8):
                for st in range(NB):
                    ps = psO.tile([P, WIN], F32, name="o_ps")
                    for dsub in range(2):
                        _mm(nc, ps[:], wo_sb[:, dsub, ec * P:(ec + 1) * P],
                            attnT_sb[:, dsub, st * WIN:(st + 1) * WIN],
                            start=(dsub == 0), stop=(dsub == 1))
                    ot = otp.tile([P, WIN], F32, name="ot")
                    nc.vector.tensor_copy(ot[:], ps[:])
                    nc.sync.dma_start(
                        outT[ec * P:(ec + 1) * P, st * WIN:(st + 1) * WIN], ot[:])

    nc.compile()
    return nc


def rnd12(a):
    """Round fp32 array to nearest float32r (12-bit mantissa)."""
    u = np.ascontiguousarray(a, np.float32).view(np.uint32)
    u = ((u.astype(np.uint64) + 0x400) & 0xFFFFF800).astype(np.uint32)
    return u.view(np.float32)


def make_in_maps(x, Wq_w, Wq_b, Wk_w, Wk_b, Wv_w, Wv_b, Wo_w, Wo_b):
    x = np.ascontiguousarray(np.asarray(x, np.float32))
    Wq8 = np.asarray(Wq_w, np.float32) / 8.0
    bq8 = np.asarray(Wq_b, np.float32) / 8.0
    wqT = rnd12(Wq8.T)
    wkT = rnd12(np.asarray(Wk_w, np.float32).T)
    wvT = rnd12(np.asarray(Wv_w, np.float32).T)
    woT = rnd12(np.asarray(Wo_w, np.float32).T)

    mask = np.zeros((NB, P, WIN), np.float32)
    for m in range(NB):
        c_idx = m * P + np.arange(P)[:, None]
        q_idx = np.arange(WIN)[None, :]
        mask[m] = (c_idx <= q_idx).astype(np.float32)

    xTb = [rnd12(x[b].T) for b in range(B)]

    in_maps = []
    for core in range(NCORES):
        b = core // 4
        h0 = (core % 4) * HPC
        dsl = slice(h0 * DK, (h0 + HPC) * DK)
        bv_core = np.asarray(Wv_b, np.float32)[dsl]
        # suffix colsum(v) table computed on host from the rounded operands:
        # colsum_j(v) = (sum_{s in block j} x[s,:]) @ WvT_core + 512*bv
        wvT_core = np.ascontiguousarray(wvT[:, dsl])
        rowsum = np.stack([xTb[b][:, j * WIN:(j + 1) * WIN].sum(axis=1)
                           for j in range(NB)])            # [NB, D]
        cs = rowsum @ wvT_core + WIN * bv_core[None, :]     # [NB, DCORE]
        sfx_full = np.zeros((NB, DCORE), np.float32)
        for bi in range(NB - 1):
            sfx_full[bi] = cs[bi + 1:].sum(axis=0)
        sfx = np.zeros((P, 2, NB), np.float32)
        for hc in range(2):
            for bi in range(NB):
                sfx[:, hc, bi] = sfx_full[bi][hc * P:(hc + 1) * P]
        in_maps.append({
            "xT": xTb[b],
            "wqT": np.ascontiguousarray(wqT[:, dsl]),
            "wkT": np.ascontiguousarray(wkT[:, dsl]),
            "wvT": np.ascontiguousarray(wvT[:, dsl]),
            "woT": np.ascontiguousarray(woT[dsl, :]),
            "bq": np.ascontiguousarray(bq8[dsl]),
            "bk": np.ascontiguousarray(np.asarray(Wk_b, np.float32)[dsl]),
            "bvr": np.ascontiguousarray(np.broadcast_to(bv_core, (P, DCORE))),
            "maskd": mask,
            "onesd": np.ones((P, P), np.float32),
            "sfxd": sfx,
        })
    return in_maps


def kernel(**inputs):
    if "nc" not in _CACHE:
        _CACHE["nc"] = build_nc()
    nc = _CACHE["nc"]
    in_maps = make_in_maps(**inputs)
    kw = {}
    if TRACE:
        kw["trace"] = True
        if TRACE_CORES is not None:
            kw["trace_cores"] = TRACE_CORES
    res = run_bass_kernel_spmd(nc, in_maps, list(range(NCORES)), **kw)
    _CACHE["last_result"] = res

    bo = np.asarray(inputs["Wo_b"], np.float32)
    out = np.zeros((B, S, D), np.float32)
    for b in range(B):
        acc = np.zeros((D, S), np.float32)
        for core in range(b * 4, b * 4 + 4):
            acc += res.results[core]["outT"]
        out[b] = acc.T + bo
    return out
